# revision 31
# baseline (speedup 1.0000x reference)
"""Causal self-attention (B=2, S=4096, D=512, H=8) on 8 Trainium2 cores.

Sharding: core c handles batch b = c//4 and heads {2*(c%4), 2*(c%4)+1}.

Design (v2): k-major flash-style attention with the exp() wall split across
TWO engines:
  - ScalarE computes exp natively (ACTIVATE, ~(N+352)/1.2 ns).
  - VectorE computes a one-instruction Schraudolph exp: writing
    int16(round(A*score + bias)) whose bit pattern IS the bf16 of
    2^(log2e*score/8 + delta): the exponent-bit trick computed directly in
    the >>16 scale.  Per-key exponent dither delta_r decorrelates the
    interpolation error; V rows (and the den ones-column) are pre-scaled by
    2^-delta_r on the host so the dither cancels exactly in PV.
A greedy ns-balancer assigns each score tile's exp (and the psum->sbuf
copies) to whichever of ACT/DVE is less loaded, so both engines run ~full
tilt alongside the TensorE stream.

Attention runs in 512-wide query chunks; projections for chunk J+1, the
output projection for chunk J-1, V transposes, and DMA are emitted as
background tasks interleaved between attention steps so PE never idles
(keeps the HAM clock at 2.4 GHz).  Denominators ride the PV matmul as a
65th 'ones' row; oT is divided on-device (reciprocal_approx_fast + gpsimd
partition broadcast) so the two heads fold into ONE output-projection pass
and the core writes a single [512, S] bf16 partial that the host sums.

PSUM budget (8 banks): pv0 pv1 | st x4 (score tiles, f32) | bg x2 (shared
by projections / V-transpose / out-projection).
"""

import sys

sys.path.insert(0, "/opt/trn_rl_repo")

from contextlib import ExitStack

import ml_dtypes
import numpy as np

import concourse.bass as bass
import concourse.tile as tile
from concourse import bacc, bass_utils, mybir

B, S, D = 2, 4096, 512
H, HD = 8, 64
NCORES = 8
F32 = mybir.dt.float32
BF16 = mybir.dt.bfloat16
I16 = mybir.dt.int16
FP8 = mybir.dt.float8e4
DR = mybir.MatmulPerfMode.DoubleRow
NPFP8 = ml_dtypes.float8_e4m3
EXP = mybir.ActivationFunctionType.Exp
IDENT = mybir.ActivationFunctionType.Identity
COPYF = mybir.ActivationFunctionType.Copy
MULT = mybir.AluOpType.mult
ADD = mybir.AluOpType.add
NPBF16 = ml_dtypes.bfloat16

CK = 512                      # query-chunk width
NCH = S // CK                 # 8
KBLK = 128                    # key block (partition dim)
KB_PER_CK = CK // KBLK        # 4
NEG = -1.0e30
LOG2E = 1.4426950408889634
A128 = 128 * LOG2E * 0.125    # DVE trick multiplier (raw-score units)
CSH = -0.045                  # Schraudolph shift
PHI = 0.6180339887498949


class Balancer:
    """Greedy ns-accounting across ACT and DVE for balanceable ops."""

    def __init__(self, nc):
        self.nc = nc
        self.ns = {"act": 2700.0, "dve": 0.0}  # ACT pays the exp table load

    def _cost(self, eng, w):
        return (w + 352) / 1.2 if eng == "act" else (w + 90) / 0.96

    def charge(self, eng, w):
        self.ns[eng] += self._cost(eng, w)

    def pick(self, w):
        eng = "act" if self.ns["act"] + self._cost("act", w) <= \
            self.ns["dve"] + self._cost("dve", w) else "dve"
        self.charge(eng, w)
        return eng


def _emit(nc, tc, ctx, io):
    xT, wpack, fpack, poT = io

    bal = Balancer(nc)

    const = ctx.enter_context(tc.tile_pool(name="const", bufs=1))
    sb = ctx.enter_context(tc.tile_pool(name="sb", bufs=1))

    # ---- constants / weights (two packed DMAs to keep the lead-in short) ----
    cb = const.tile([128, 2240], BF16, tag="cbf16")
    cf = const.tile([128, 68], F32, tag="cf32")
    nc.sync.dma_start(cb[:], wpack[:])
    nc.sync.dma_start(cf[:], fpack[:])
    W_Q, W_K, W_V, W_O, TRI, ID2 = 0, 512, 1024, 1536, 2048, 2176
    BQKV, KB23, BACT, VSC = 0, 3, 35, 67

    # ---- persistent SBUF ----
    kT = sb.tile([128, S], BF16, tag="kT")       # [2*64 hd, keys]
    v0 = sb.tile([128, 32 * 65], BF16, tag="v0")  # k-major V + ones col, h0
    v1 = sb.tile([128, 32 * 65], BF16, tag="v1")

    xin = ctx.enter_context(tc.tile_pool(name="xin", bufs=2))
    qp = ctx.enter_context(tc.tile_pool(name="qp", bufs=2))
    vtp = ctx.enter_context(tc.tile_pool(name="vtp", bufs=2))
    etp = ctx.enter_context(tc.tile_pool(name="etp", bufs=6))
    otp = ctx.enter_context(tc.tile_pool(name="otp", bufs=2))
    pop = ctx.enter_context(tc.tile_pool(name="pop", bufs=4))
    rdp = ctx.enter_context(tc.tile_pool(name="rdp", bufs=2))

    ps_pv = ctx.enter_context(tc.tile_pool(name="ps_pv", bufs=1, space="PSUM"))
    ps_st = ctx.enter_context(tc.tile_pool(name="ps_st", bufs=1, space="PSUM"))
    ps_bg = ctx.enter_context(tc.tile_pool(name="ps_bg", bufs=2, space="PSUM"))

    # ones columns of v0/v1 (scaled 2^-delta); written once, blocks fill later
    for vdst in (v0, v1):
        ones_col = vdst[:].rearrange("p (k c) -> p k c", c=65)[:, :, 64:65]
        nc.vector.tensor_copy(ones_col, cf[:, VSC:VSC + 1].to_broadcast((128, 32, 1)))

    # ---------------- background task machinery ----------------
    bg_tasks = []
    pace = {"credit": 0.0, "rate": 1.0}

    def drain(n):
        for _ in range(min(n, len(bg_tasks))):
            bg_tasks.pop(0)()

    def drain_paced():
        """Spread queued tasks over the chunk's drain slots so the PE always
        has background work, even late in a chunk."""
        pace["credit"] += pace["rate"]
        while pace["credit"] >= 1.0 and bg_tasks:
            pace["credit"] -= 1.0
            bg_tasks.pop(0)()

    def copy_psum(dst_ap, src_ap, w, bias_col=None, scale=1.0):
        """psum->sbuf evacuation on the less-loaded of ACT/DVE."""
        eng = bal.pick(w)
        if eng == "act":
            if bias_col is not None:
                nc.scalar.activation(dst_ap, src_ap, IDENT, bias=bias_col,
                                     scale=scale)
            else:
                nc.scalar.copy(dst_ap, src_ap)
        else:
            if bias_col is not None:
                nc.vector.tensor_scalar(dst_ap, src_ap, scale, bias_col,
                                        MULT, ADD)
            else:
                nc.vector.tensor_copy(dst_ap, src_ap)

    q_tiles = {}

    def emit_proj(J):
        """q/k/v projections for chunk J + V transpose to k-major."""
        xt = xin.tile([128, 4 * CK], BF16, tag="x")
        xt3 = xt[:].rearrange("p (ks c) -> p ks c", ks=4)
        nc.sync.dma_start(
            xt3, xT[:].rearrange("(ks p) s -> p ks s", p=128)
            [:, :, J * CK:(J + 1) * CK])
        qt = qp.tile([128, CK], BF16, tag="q")
        q_tiles[J] = qt
        vt = vtp.tile([128, CK], BF16, tag="v")
        csl = slice(J * CK, (J + 1) * CK)

        def mk_proj(woff, bcol, dst_ap):
            def f():
                ps = ps_bg.tile([128, CK], F32, tag="bg")
                for ks in range(4):
                    nc.tensor.matmul(
                        ps[:], cb[:, woff + ks * 128:woff + (ks + 1) * 128],
                        xt[:, ks * CK:(ks + 1) * CK],
                        start=(ks == 0), stop=(ks == 3))
                copy_psum(dst_ap, ps[:], CK,
                          bias_col=cf[:, BQKV + bcol:BQKV + bcol + 1])
            return f

        bg_tasks.append(mk_proj(W_Q, 0, qt[:]))
        bg_tasks.append(mk_proj(W_K, 1, kT[:, csl]))
        bg_tasks.append(mk_proj(W_V, 2, vt[:]))

        def mk_vtrans(hh, vdst):
            def f():
                # own psum buffer per head: a shared bank would let head0's
                # DVE copy (bank read) overlap head1's PE transposes (bank
                # write) -> fatal PSUM collision
                ps = ps_bg.tile([128, CK], F32, tag="bg")
                tr = ps[:].bitcast(BF16)  # [128, 1024] bf16 view
                for i in range(4):
                    nc.tensor.transpose(
                        tr[:, i * 64:(i + 1) * 64],
                        vt[hh * 64:(hh + 1) * 64, i * KBLK:(i + 1) * KBLK],
                        cb[hh * 64:(hh + 1) * 64, ID2:ID2 + 64])
                dst = vdst[:, (J * 4) * 65:(J * 4 + 4) * 65]
                dst = dst.rearrange("p (k c) -> p k c", c=65)[:, :, 0:64]
                nc.vector.tensor_scalar_mul(
                    dst, tr[:, 0:256].rearrange("p (k c) -> p k c", c=64),
                    cf[:, VSC:VSC + 1])
                bal.charge("dve", 256)
            return f
        bg_tasks.append(mk_vtrans(0, v0))
        bg_tasks.append(mk_vtrans(1, v1))

    oT_tiles = {}

    def emit_div(J):
        """INLINE at chunk-J end: den reciprocal + broadcast + oT divide.
        Reads the pv psum tiles, so must precede the next pv acquisition."""
        oT = otp.tile([128, CK], BF16, tag="oT")
        oT_tiles[J] = oT
        pv0t, pv1t = pv_tiles.pop(J)
        rdB = []
        for hh, pvt in ((0, pv0t), (1, pv1t)):
            den = rdp.tile([1, CK], F32, tag=f"den{hh}")
            nc.vector.tensor_copy(den[:], pvt[64:65, :])
            rd = rdp.tile([1, CK], F32, tag=f"rd{hh}")
            nc.vector.reciprocal_approx_fast(rd[:], den[:])
            bal.charge("dve", 2 * CK)
            rb = rdp.tile([64, CK], F32, tag=f"rdB{hh}")
            nc.gpsimd.partition_broadcast(rb[:], rd[:], channels=64)
            rdB.append(rb)
        for hh, pvt in ((0, pv0t), (1, pv1t)):
            hsl = slice(hh * 64, (hh + 1) * 64)
            nc.vector.tensor_mul(oT[hsl, :], pvt[0:64, :], rdB[hh][:])
            bal.charge("dve", CK)

    def emit_outproj(J):
        """Queue chunk J's Wo matmuls + output DMA (oT(J) long ready by the
        time these drain, so they never block the PE FIFO)."""
        oT = oT_tiles.pop(J)

        def mk_dblk(dt_):
            def f():
                ps = ps_bg.tile([128, CK], F32, tag="bg")
                nc.tensor.matmul(ps[:], cb[:, W_O + dt_ * 128:W_O + (dt_ + 1) * 128],
                                 oT[:], start=True, stop=True)
                po = pop.tile([128, CK], BF16, tag="po")
                copy_psum(po[:], ps[:], CK)
                nc.sync.dma_start(
                    poT[dt_ * 128:(dt_ + 1) * 128, J * CK:(J + 1) * CK],
                    po[:])
            return f
        for dt_ in range(4):
            bg_tasks.append(mk_dblk(dt_))

    # ---------------- main pipeline ----------------
    pv_tiles = {}
    emit_proj(0)
    drain(99)  # chunk 0 projections up front

    for J in range(NCH):
        if J + 1 < NCH:
            emit_proj(J + 1)
        if J >= 1:
            emit_outproj(J - 1)  # behind proj(J+1) tasks: oT(J-1) is ready
        pv0t = ps_pv.tile([65, CK], F32, tag="pv0")
        pv1t = ps_pv.tile([65, CK], F32, tag="pv1")
        pv_tiles[J] = (pv0t, pv1t)
        nkb = KB_PER_CK * (J + 1)
        qt = q_tiles.pop(J)
        pace["rate"] = (len(bg_tasks) + 1) / (2.0 * nkb)

        def emit_pv(kb):
            p = kb - KB_PER_CK * J
            col0 = KBLK * p if p >= 0 else 0
            for hh, vsb, pv in ((0, v0, pv0t), (1, v1, pv1t)):
                nc.tensor.matmul(
                    pv[:, col0:], vsb[:, kb * 65:(kb + 1) * 65],
                    kb_ets[kb][hh][:, col0:],
                    start=(kb == 0), stop=(kb == nkb - 1))
            del kb_ets[kb]

        kb_ets = {}
        for kb in range(nkb):
            p = kb - KB_PER_CK * J
            col0 = KBLK * p if p >= 0 else 0
            w = CK - col0
            sts, ets = [], []
            for hh in range(2):
                st = ps_st.tile([128, CK], F32, tag=f"st{hh}{kb % 2}")
                hsl = slice(hh * 64, (hh + 1) * 64)
                nc.tensor.matmul(
                    st[:, col0:], kT[hsl, kb * KBLK:(kb + 1) * KBLK],
                    qt[hsl, col0:], start=True, stop=True)
                sts.append(st)
            drain_paced()
            for hh in range(2):
                et = etp.tile([128, CK], BF16, tag="et")
                eng = bal.pick(w)
                if eng == "act":
                    nc.scalar.activation(
                        et[:, col0:], sts[hh][:, col0:], EXP,
                        bias=cf[:, BACT + kb:BACT + kb + 1], scale=0.125)
                else:
                    nc.vector.tensor_scalar(
                        et[:, col0:].bitcast(I16), sts[hh][:, col0:],
                        float(A128), cf[:, KB23 + kb:KB23 + kb + 1],
                        MULT, ADD)
                if p >= 0:
                    nc.vector.tensor_mul(
                        et[:, col0:col0 + KBLK], et[:, col0:col0 + KBLK],
                        cb[:, TRI:TRI + 128])
                    bal.charge("dve", KBLK / 2)
                ets.append(et)
            kb_ets[kb] = ets
            # software pipeline: PV(kb-1) sits BEHIND QK(kb) in the PE FIFO
            # so the PE streams QK(kb) while exp(kb-1) finishes
            if kb >= 1:
                emit_pv(kb - 1)
            drain_paced()
        emit_pv(nkb - 1)
        emit_div(J)
        drain(1)

    emit_outproj(NCH - 1)
    drain(99)


_CACHED = None


def _build():
    global _CACHED
    if _CACHED is not None:
        return _CACHED
    nc = bacc.Bacc("TRN2", target_bir_lowering=False, debug=False,
                   enable_asserts=False, num_devices=NCORES)
    names = [
        ("xT", [D, S], BF16), ("wpack", [128, 2240], BF16),
        ("fpack", [128, 68], F32),
    ]
    aps = [nc.dram_tensor(n, sh, dt_, kind="ExternalInput").ap()
           for n, sh, dt_ in names]
    poT = nc.dram_tensor("poT", [D, S], BF16, kind="ExternalOutput").ap()
    with tile.TileContext(nc) as tc, ExitStack() as ctx:
        _emit(nc, tc, ctx, aps + [poT])
    nc.compile()
    _CACHED = nc
    return nc


def _host_inputs(x, attention_mask, Wq, bq, Wk, bk, Wv, bv, Wo, bo):
    f = np.float32
    x = np.asarray(x, f)
    mask = np.asarray(attention_mask)
    Wq, Wk, Wv, Wo = (np.asarray(w, f) for w in (Wq, Wk, Wv, Wo))
    bq, bk, bv = (np.asarray(b_, f) for b_ in (bq, bk, bv))
    tri = np.triu(np.ones((128, 128), NPBF16))      # [k,q]: 1 where q >= k
    id2 = np.tile(np.eye(64, dtype=NPBF16), (2, 1))
    delta = ((np.arange(128) * PHI) % 1.0).astype(f)          # per key%128
    vscale = (2.0 ** -delta)[:, None].astype(f)
    in_maps = []
    for c in range(NCORES):
        b = c // 4
        h0 = 2 * (c % 4)
        hsl = slice(64 * h0, 64 * h0 + 128)

        def pack_w(W):
            wt = W[hsl, :].T                        # [512, 128] = Wh^T
            return np.ascontiguousarray(
                wt.reshape(4, 128, 128).transpose(1, 0, 2)
                .reshape(128, 512).astype(NPBF16))

        wo_t = Wo[:, hsl].T.astype(NPBF16)           # [128, 512]
        mk = np.where(mask[b] != 0, f(0.0), f(NEG)).astype(f)  # [S]
        mk = mk.reshape(32, 128).T                   # [128 part, 32 kb]
        kb23 = (128.0 * (127.0 + CSH) + 128.0 * delta)[:, None] + \
            np.where(mk < 0, f(-1e9), f(0.0))
        biasact = (delta * np.log(2.0))[:, None] + mk
        wpack = np.concatenate(
            [pack_w(Wq), pack_w(Wk), pack_w(Wv), wo_t, tri, id2], axis=1)
        fpack = np.concatenate(
            [np.stack([bq[hsl], bk[hsl], bv[hsl]], axis=1).astype(f),
             kb23.astype(f), biasact.astype(f), vscale], axis=1)

        in_maps.append({
            "xT": np.ascontiguousarray(x[b].T.astype(NPBF16)),
            "wpack": np.ascontiguousarray(wpack),
            "fpack": np.ascontiguousarray(fpack),
        })
    return in_maps


def _assemble(results, bo):
    out = np.zeros((B, S, D), np.float32)
    for c in range(NCORES):
        out[c // 4] += results[c]["poT"].astype(np.float32).T
    out += np.asarray(bo, np.float32)
    return out


def kernel(**inputs) -> np.ndarray:
    nc = _build()
    in_maps = _host_inputs(**inputs)
    last_err = None
    for attempt in range(3):
        try:
            res = bass_utils.run_bass_kernel_spmd(
                nc, in_maps, core_ids=list(range(NCORES)))
            out = _assemble(res.results, inputs["bo"])
        except Exception as e:  # transient NRT/axon device errors
            last_err = e
            continue
        if np.isfinite(out).all():
            return out
        last_err = RuntimeError("non-finite output")
    raise last_err


def run_traced(inputs, **kwargs):
    """test.py helper: run with NTFF tracing, return (out, BassKernelResults)."""
    nc = _build()
    in_maps = _host_inputs(**inputs)
    res = bass_utils.run_bass_kernel_spmd(
        nc, in_maps, core_ids=list(range(NCORES)), trace=True, **kwargs)
    return _assemble(res.results, inputs["bo"]), res


# revision 32
# speedup vs baseline: 1.0132x; 1.0132x over previous
"""Causal self-attention (B=2, S=4096, D=512, H=8) on 8 Trainium2 cores.

Sharding: core c handles batch b = c//4 and heads {2*(c%4), 2*(c%4)+1}.

Design (v2): k-major flash-style attention with the exp() wall split across
TWO engines:
  - ScalarE computes exp natively (ACTIVATE, ~(N+352)/1.2 ns).
  - VectorE computes a one-instruction Schraudolph exp: writing
    int16(round(A*score + bias)) whose bit pattern IS the bf16 of
    2^(log2e*score/8 + delta): the exponent-bit trick computed directly in
    the >>16 scale.  Per-key exponent dither delta_r decorrelates the
    interpolation error; V rows (and the den ones-column) are pre-scaled by
    2^-delta_r on the host so the dither cancels exactly in PV.
A greedy ns-balancer assigns each score tile's exp (and the psum->sbuf
copies) to whichever of ACT/DVE is less loaded, so both engines run ~full
tilt alongside the TensorE stream.

Attention runs in 512-wide query chunks; projections for chunk J+1, the
output projection for chunk J-1, V transposes, and DMA are emitted as
background tasks interleaved between attention steps so PE never idles
(keeps the HAM clock at 2.4 GHz).  Denominators ride the PV matmul as a
65th 'ones' row; oT is divided on-device (reciprocal_approx_fast + gpsimd
partition broadcast) so the two heads fold into ONE output-projection pass
and the core writes a single [512, S] bf16 partial that the host sums.

PSUM budget (8 banks): pv0 pv1 | st x4 (score tiles, f32) | bg x2 (shared
by projections / V-transpose / out-projection).
"""

import sys

sys.path.insert(0, "/opt/trn_rl_repo")

from contextlib import ExitStack

import ml_dtypes
import numpy as np

import concourse.bass as bass
import concourse.tile as tile
from concourse import bacc, bass_utils, mybir

B, S, D = 2, 4096, 512
H, HD = 8, 64
NCORES = 8
F32 = mybir.dt.float32
BF16 = mybir.dt.bfloat16
I16 = mybir.dt.int16
FP8 = mybir.dt.float8e4
DR = mybir.MatmulPerfMode.DoubleRow
NPFP8 = ml_dtypes.float8_e4m3
EXP = mybir.ActivationFunctionType.Exp
IDENT = mybir.ActivationFunctionType.Identity
COPYF = mybir.ActivationFunctionType.Copy
MULT = mybir.AluOpType.mult
ADD = mybir.AluOpType.add
NPBF16 = ml_dtypes.bfloat16

CK = 512                      # query-chunk width
NCH = S // CK                 # 8
KBLK = 128                    # key block (partition dim)
KB_PER_CK = CK // KBLK        # 4
NEG = -1.0e30
LOG2E = 1.4426950408889634
A128 = 128 * LOG2E * 0.125    # DVE trick multiplier (raw-score units)
CSH = -0.045                  # Schraudolph shift
PHI = 0.6180339887498949


class Balancer:
    """Greedy ns-accounting across ACT and DVE for balanceable ops."""

    def __init__(self, nc):
        self.nc = nc
        self.ns = {"act": 2700.0, "dve": 0.0}  # ACT pays the exp table load

    def _cost(self, eng, w):
        return (w + 352) / 1.2 if eng == "act" else (w + 90) / 0.96

    def charge(self, eng, w):
        self.ns[eng] += self._cost(eng, w)

    def pick(self, w):
        eng = "act" if self.ns["act"] + self._cost("act", w) <= \
            self.ns["dve"] + self._cost("dve", w) else "dve"
        self.charge(eng, w)
        return eng


def _emit(nc, tc, ctx, io):
    xT, wpack, fpack, poT = io

    bal = Balancer(nc)

    const = ctx.enter_context(tc.tile_pool(name="const", bufs=1))
    sb = ctx.enter_context(tc.tile_pool(name="sb", bufs=1))

    # ---- constants / weights (two packed DMAs to keep the lead-in short) ----
    cb = const.tile([128, 2240], BF16, tag="cbf16")
    cf = const.tile([128, 68], F32, tag="cf32")
    nc.sync.dma_start(cb[:], wpack[:])
    nc.sync.dma_start(cf[:], fpack[:])
    W_Q, W_K, W_V, W_O, TRI, ID2 = 0, 512, 1024, 1536, 2048, 2176
    BQKV, KB23, BACT, VSC = 0, 3, 35, 67

    # ---- persistent SBUF ----
    kT = sb.tile([128, S], BF16, tag="kT")       # [2*64 hd, keys]
    v0 = sb.tile([128, 32 * 65], BF16, tag="v0")  # k-major V + ones col, h0
    v1 = sb.tile([128, 32 * 65], BF16, tag="v1")

    xin = ctx.enter_context(tc.tile_pool(name="xin", bufs=2))
    qp = ctx.enter_context(tc.tile_pool(name="qp", bufs=2))
    vtp = ctx.enter_context(tc.tile_pool(name="vtp", bufs=2))
    etp = ctx.enter_context(tc.tile_pool(name="etp", bufs=6))
    otp = ctx.enter_context(tc.tile_pool(name="otp", bufs=2))
    pop = ctx.enter_context(tc.tile_pool(name="pop", bufs=4))
    rdp = ctx.enter_context(tc.tile_pool(name="rdp", bufs=2))

    ps_pv = ctx.enter_context(tc.tile_pool(name="ps_pv", bufs=1, space="PSUM"))
    ps_st = ctx.enter_context(tc.tile_pool(name="ps_st", bufs=1, space="PSUM"))
    ps_bg = ctx.enter_context(tc.tile_pool(name="ps_bg", bufs=2, space="PSUM"))

    # ones columns of v0/v1 (scaled 2^-delta); written once, blocks fill later
    for vdst in (v0, v1):
        ones_col = vdst[:].rearrange("p (k c) -> p k c", c=65)[:, :, 64:65]
        nc.vector.tensor_copy(ones_col, cf[:, VSC:VSC + 1].to_broadcast((128, 32, 1)))

    # ---------------- background task machinery ----------------
    bg_tasks = []
    pace = {"credit": 0.0, "rate": 1.0}

    def drain(n):
        for _ in range(min(n, len(bg_tasks))):
            bg_tasks.pop(0)()

    def drain_paced():
        """Spread queued tasks over the chunk's drain slots so the PE always
        has background work, even late in a chunk."""
        pace["credit"] += pace["rate"]
        while pace["credit"] >= 1.0 and bg_tasks:
            pace["credit"] -= 1.0
            bg_tasks.pop(0)()

    def copy_psum(dst_ap, src_ap, w, bias_col=None, scale=1.0):
        """psum->sbuf evacuation on the less-loaded of ACT/DVE."""
        eng = bal.pick(w)
        if eng == "act":
            if bias_col is not None:
                nc.scalar.activation(dst_ap, src_ap, IDENT, bias=bias_col,
                                     scale=scale)
            else:
                nc.scalar.copy(dst_ap, src_ap)
        else:
            if bias_col is not None:
                nc.vector.tensor_scalar(dst_ap, src_ap, scale, bias_col,
                                        MULT, ADD)
            else:
                nc.vector.tensor_copy(dst_ap, src_ap)

    q_tiles = {}

    def emit_proj(J):
        """q/k/v projections for chunk J + V transpose to k-major."""
        xt = xin.tile([128, 4 * CK], BF16, tag="x")
        for ks in range(4):
            nc.sync.dma_start(
                xt[:, ks * CK:(ks + 1) * CK],
                xT[ks * 128:(ks + 1) * 128, J * CK:(J + 1) * CK])
        qt = qp.tile([128, CK], BF16, tag="q")
        q_tiles[J] = qt
        vt = vtp.tile([128, CK], BF16, tag="v")
        csl = slice(J * CK, (J + 1) * CK)

        def mk_proj(woff, bcol, dst_ap):
            def f():
                ps = ps_bg.tile([128, CK], F32, tag="bg")
                for ks in range(4):
                    nc.tensor.matmul(
                        ps[:], cb[:, woff + ks * 128:woff + (ks + 1) * 128],
                        xt[:, ks * CK:(ks + 1) * CK],
                        start=(ks == 0), stop=(ks == 3))
                copy_psum(dst_ap, ps[:], CK,
                          bias_col=cf[:, BQKV + bcol:BQKV + bcol + 1])
            return f

        bg_tasks.append(mk_proj(W_Q, 0, qt[:]))
        bg_tasks.append(mk_proj(W_K, 1, kT[:, csl]))
        bg_tasks.append(mk_proj(W_V, 2, vt[:]))

        def mk_vtrans(hh, vdst):
            def f():
                # own psum buffer per head: a shared bank would let head0's
                # DVE copy (bank read) overlap head1's PE transposes (bank
                # write) -> fatal PSUM collision
                ps = ps_bg.tile([128, CK], F32, tag="bg")
                tr = ps[:].bitcast(BF16)  # [128, 1024] bf16 view
                for i in range(4):
                    nc.tensor.transpose(
                        tr[:, i * 64:(i + 1) * 64],
                        vt[hh * 64:(hh + 1) * 64, i * KBLK:(i + 1) * KBLK],
                        cb[hh * 64:(hh + 1) * 64, ID2:ID2 + 64])
                dst = vdst[:, (J * 4) * 65:(J * 4 + 4) * 65]
                dst = dst.rearrange("p (k c) -> p k c", c=65)[:, :, 0:64]
                nc.vector.tensor_scalar_mul(
                    dst, tr[:, 0:256].rearrange("p (k c) -> p k c", c=64),
                    cf[:, VSC:VSC + 1])
                bal.charge("dve", 256)
            return f
        bg_tasks.append(mk_vtrans(0, v0))
        bg_tasks.append(mk_vtrans(1, v1))

    oT_tiles = {}

    def emit_div(J):
        """INLINE at chunk-J end: den reciprocal + broadcast + oT divide.
        Reads the pv psum tiles, so must precede the next pv acquisition."""
        oT = otp.tile([128, CK], BF16, tag="oT")
        oT_tiles[J] = oT
        pv0t, pv1t = pv_tiles.pop(J)
        rdB = []
        for hh, pvt in ((0, pv0t), (1, pv1t)):
            den = rdp.tile([1, CK], F32, tag=f"den{hh}")
            nc.vector.tensor_copy(den[:], pvt[64:65, :])
            rd = rdp.tile([1, CK], F32, tag=f"rd{hh}")
            nc.vector.reciprocal_approx_fast(rd[:], den[:])
            bal.charge("dve", 2 * CK)
            rb = rdp.tile([64, CK], F32, tag=f"rdB{hh}")
            nc.gpsimd.partition_broadcast(rb[:], rd[:], channels=64)
            rdB.append(rb)
        for hh, pvt in ((0, pv0t), (1, pv1t)):
            hsl = slice(hh * 64, (hh + 1) * 64)
            nc.vector.tensor_mul(oT[hsl, :], pvt[0:64, :], rdB[hh][:])
            bal.charge("dve", CK)

    def emit_outproj(J):
        """Queue chunk J's Wo matmuls + output DMA (oT(J) long ready by the
        time these drain, so they never block the PE FIFO)."""
        oT = oT_tiles.pop(J)

        def mk_dblk(dt_):
            def f():
                ps = ps_bg.tile([128, CK], F32, tag="bg")
                nc.tensor.matmul(ps[:], cb[:, W_O + dt_ * 128:W_O + (dt_ + 1) * 128],
                                 oT[:], start=True, stop=True)
                po = pop.tile([128, CK], BF16, tag="po")
                copy_psum(po[:], ps[:], CK)
                nc.sync.dma_start(
                    poT[dt_ * 128:(dt_ + 1) * 128, J * CK:(J + 1) * CK],
                    po[:])
            return f
        for dt_ in range(4):
            bg_tasks.append(mk_dblk(dt_))

    # ---------------- main pipeline ----------------
    pv_tiles = {}
    emit_proj(0)
    drain(99)  # chunk 0 projections up front

    for J in range(NCH):
        if J + 1 < NCH:
            emit_proj(J + 1)
        if J >= 1:
            emit_outproj(J - 1)  # behind proj(J+1) tasks: oT(J-1) is ready
        pv0t = ps_pv.tile([65, CK], F32, tag="pv0")
        pv1t = ps_pv.tile([65, CK], F32, tag="pv1")
        pv_tiles[J] = (pv0t, pv1t)
        nkb = KB_PER_CK * (J + 1)
        qt = q_tiles.pop(J)
        pace["rate"] = (len(bg_tasks) + 1) / (2.0 * nkb)

        def emit_pv(kb):
            p = kb - KB_PER_CK * J
            col0 = KBLK * p if p >= 0 else 0
            for hh, vsb, pv in ((0, v0, pv0t), (1, v1, pv1t)):
                nc.tensor.matmul(
                    pv[:, col0:], vsb[:, kb * 65:(kb + 1) * 65],
                    kb_ets[kb][hh][:, col0:],
                    start=(kb == 0), stop=(kb == nkb - 1))
            del kb_ets[kb]

        kb_ets = {}
        for kb in range(nkb):
            p = kb - KB_PER_CK * J
            col0 = KBLK * p if p >= 0 else 0
            w = CK - col0
            sts, ets = [], []
            for hh in range(2):
                st = ps_st.tile([128, CK], F32, tag=f"st{hh}{kb % 2}")
                hsl = slice(hh * 64, (hh + 1) * 64)
                nc.tensor.matmul(
                    st[:, col0:], kT[hsl, kb * KBLK:(kb + 1) * KBLK],
                    qt[hsl, col0:], start=True, stop=True)
                sts.append(st)
            drain_paced()
            for hh in range(2):
                et = etp.tile([128, CK], BF16, tag="et")
                eng = bal.pick(w)
                if eng == "act":
                    nc.scalar.activation(
                        et[:, col0:], sts[hh][:, col0:], EXP,
                        bias=cf[:, BACT + kb:BACT + kb + 1], scale=0.125)
                else:
                    nc.vector.tensor_scalar(
                        et[:, col0:].bitcast(I16), sts[hh][:, col0:],
                        float(A128), cf[:, KB23 + kb:KB23 + kb + 1],
                        MULT, ADD)
                if p >= 0:
                    nc.vector.tensor_mul(
                        et[:, col0:col0 + KBLK], et[:, col0:col0 + KBLK],
                        cb[:, TRI:TRI + 128])
                    bal.charge("dve", KBLK / 2)
                ets.append(et)
            kb_ets[kb] = ets
            # software pipeline: PV(kb-1) sits BEHIND QK(kb) in the PE FIFO
            # so the PE streams QK(kb) while exp(kb-1) finishes
            if kb >= 1:
                emit_pv(kb - 1)
            drain_paced()
        emit_pv(nkb - 1)
        emit_div(J)
        drain(1)

    emit_outproj(NCH - 1)
    drain(99)


_CACHED = None


def _build():
    global _CACHED
    if _CACHED is not None:
        return _CACHED
    nc = bacc.Bacc("TRN2", target_bir_lowering=False, debug=False,
                   enable_asserts=False, num_devices=NCORES)
    names = [
        ("xT", [D, S], BF16), ("wpack", [128, 2240], BF16),
        ("fpack", [128, 68], F32),
    ]
    aps = [nc.dram_tensor(n, sh, dt_, kind="ExternalInput").ap()
           for n, sh, dt_ in names]
    poT = nc.dram_tensor("poT", [D, S], BF16, kind="ExternalOutput").ap()
    with tile.TileContext(nc) as tc, ExitStack() as ctx:
        _emit(nc, tc, ctx, aps + [poT])
    nc.compile()
    _CACHED = nc
    return nc


def _host_inputs(x, attention_mask, Wq, bq, Wk, bk, Wv, bv, Wo, bo):
    f = np.float32
    x = np.asarray(x, f)
    mask = np.asarray(attention_mask)
    Wq, Wk, Wv, Wo = (np.asarray(w, f) for w in (Wq, Wk, Wv, Wo))
    bq, bk, bv = (np.asarray(b_, f) for b_ in (bq, bk, bv))
    tri = np.triu(np.ones((128, 128), NPBF16))      # [k,q]: 1 where q >= k
    id2 = np.tile(np.eye(64, dtype=NPBF16), (2, 1))
    delta = ((np.arange(128) * PHI) % 1.0).astype(f)          # per key%128
    vscale = (2.0 ** -delta)[:, None].astype(f)
    in_maps = []
    for c in range(NCORES):
        b = c // 4
        h0 = 2 * (c % 4)
        hsl = slice(64 * h0, 64 * h0 + 128)

        def pack_w(W):
            wt = W[hsl, :].T                        # [512, 128] = Wh^T
            return np.ascontiguousarray(
                wt.reshape(4, 128, 128).transpose(1, 0, 2)
                .reshape(128, 512).astype(NPBF16))

        wo_t = Wo[:, hsl].T.astype(NPBF16)           # [128, 512]
        mk = np.where(mask[b] != 0, f(0.0), f(NEG)).astype(f)  # [S]
        mk = mk.reshape(32, 128).T                   # [128 part, 32 kb]
        kb23 = (128.0 * (127.0 + CSH) + 128.0 * delta)[:, None] + \
            np.where(mk < 0, f(-1e9), f(0.0))
        biasact = (delta * np.log(2.0))[:, None] + mk
        wpack = np.concatenate(
            [pack_w(Wq), pack_w(Wk), pack_w(Wv), wo_t, tri, id2], axis=1)
        fpack = np.concatenate(
            [np.stack([bq[hsl], bk[hsl], bv[hsl]], axis=1).astype(f),
             kb23.astype(f), biasact.astype(f), vscale], axis=1)

        in_maps.append({
            "xT": np.ascontiguousarray(x[b].T.astype(NPBF16)),
            "wpack": np.ascontiguousarray(wpack),
            "fpack": np.ascontiguousarray(fpack),
        })
    return in_maps


def _assemble(results, bo):
    out = np.zeros((B, S, D), np.float32)
    for c in range(NCORES):
        out[c // 4] += results[c]["poT"].astype(np.float32).T
    out += np.asarray(bo, np.float32)
    return out


def kernel(**inputs) -> np.ndarray:
    nc = _build()
    in_maps = _host_inputs(**inputs)
    last_err = None
    for attempt in range(3):
        try:
            res = bass_utils.run_bass_kernel_spmd(
                nc, in_maps, core_ids=list(range(NCORES)))
            out = _assemble(res.results, inputs["bo"])
        except Exception as e:  # transient NRT/axon device errors
            last_err = e
            continue
        if np.isfinite(out).all():
            return out
        last_err = RuntimeError("non-finite output")
    raise last_err


def run_traced(inputs, **kwargs):
    """test.py helper: run with NTFF tracing, return (out, BassKernelResults)."""
    nc = _build()
    in_maps = _host_inputs(**inputs)
    res = bass_utils.run_bass_kernel_spmd(
        nc, in_maps, core_ids=list(range(NCORES)), trace=True, **kwargs)
    return _assemble(res.results, inputs["bo"]), res
